# revision 50
# baseline (speedup 1.0000x reference)
"""Multi-head attention (N=4, T=2048, D=512, H=8, dh=64) on 8 TRN2 NeuronCores.

Sharding: batch N (4) x head-group (2 groups of 4 heads) -> 8 cores.
Per core, for its (batch n, head-group g) the kernel computes
  q = query[n] @ Wq[:, 256g:+256], k/v likewise, then per head
  softmax(q k^T / sqrt(512)) v, assembled host-side from oT tiles.

Implementation notes:
- Score matmuls (contraction dh=64) alternate the two heads of a pair
  between PE row-tiles (0,0)/(64,0); the tiles stream concurrently
  (~227ns per matmul pair vs ~490ns/matmul same-tile, HW-measured).
- q/k projections run as fp8(e4m3) DoubleRow matmuls: contraction 512 is
  consumed 256/pass (two interleaved 128-row slabs), halving the chain.
  Only the scores depend on q/k, and softmax normalization makes the
  score path tolerant of fp8 (|s|<0.8, exp flat). V stays bf16.
- exp() is split across ScalarE ACTIVATE (true exp) and a custom DVE op
  evaluating a minimax cubic (c0=1, rel err 6e-3 on +-0.9; scores are
  tiny so no max-subtraction is needed and the cubic stays positive).
- PSUM: three 2-bank score pools rotate (depth 3) so the score matmuls
  never wait on a single exp; 16 one-ktile groups per (q-block,
  head-pair) slot, engines interleaved S/D. The previous slot's AV
  matmuls (bf16, ones-column for denominators) fill the gaps at a fixed
  cadence, its normalize runs right after the last AV chunk, and the
  final slot drains its own AV inline to shorten the tail.
"""

import math

import ml_dtypes
import numpy as np

import concourse.bass as bass
import concourse.mybir as mybir
import concourse.tile as tile
from concourse import bacc
from concourse.bass_utils import run_bass_kernel_spmd

# ---- custom DVE op: out = 1 + x(c1 + x(c2 + x c3)) ------------------------
import concourse.dve_ops as dve_ops
from concourse.dve_spec import Spec, Src0, C0, C1, C2, One, lower
from concourse.dve_uop import DveOpSpec


def _register_exp_op():
    name = "EXP_POLY3_ANT"
    for o in dve_ops.OPS:
        if o.name == name:
            return o
    body = ((Src0 * C0 + C1) * Src0 + C2) * Src0 + One
    spec = Spec(
        body=body,
        reference=lambda in0, s0, s1, imm2: ((in0 * s0 + s1) * in0 + imm2) * in0
        + 1.0,
    )
    row = dve_ops._CUSTOM_DVE_ROW_BASE + len(dve_ops.OPS)
    shas = {}
    for ver in ("v3", "v4"):
        try:
            uops = lower(spec, ver=ver)
            shas[ver] = DveOpSpec(
                name=name, opcode=row, uops=uops, rd1_en=False
            ).sha(ver)
        except Exception:
            pass
    op = dve_ops.DveOp(name, spec, subdim=False, uops_sha=shas)
    dve_ops.OPS.append(op)
    dve_ops._SUB_OPCODE_FOR_NAME[name] = row
    dve_ops.CUSTOM_DVE_SPECS[name] = spec
    return op


EXP_OP = _register_exp_op()

F32 = mybir.dt.float32
BF16 = mybir.dt.bfloat16
FP8 = mybir.dt.float8e4
EXP = mybir.ActivationFunctionType.Exp
DR = mybir.MatmulPerfMode.DoubleRow

N, T, D = 4, 2048, 512
HPC, DH = 4, 64          # heads per core, head dim
GC = HPC * DH            # head-group columns (256)
SCALE = 1.0 / math.sqrt(D)
QB = 512                 # q block
NQB = T // QB            # 4
NKT = T // 128           # 16 k tiles
KS = D // 128            # 4 contraction slices for projections

# minimax cubic for exp on [-0.9, 0.9], c0 = 1; coeffs pre-scaled by SCALE^i
_C1, _C2, _C3 = 1.0122206024824583, 0.5302855202358088, 0.15680354230475546
PC1, PC2, PC3 = _C1 * SCALE, _C2 * SCALE**2, _C3 * SCALE**3

# per-slot exp engine pattern over the 16 one-ktile groups (S=ScalarE true
# exp, D=DVE cubic); 10 S / 6 D balances the two engines' throughput.
ENGINES = "SDSSDSDSSDSDSSDS"
# AV chunks for the previous slot, emitted before the group at these
# indices (the chunk at 0 covers the slot-start pool/exp lag).
AV_BEFORE = {0: (0, 4), 3: (4, 7), 6: (7, 10), 9: (10, 13), 12: (13, 16)}


def build():
    nc = bacc.Bacc("TRN2", target_bir_lowering=False, debug=False, num_devices=8)
    kT_in = nc.declare_dram_parameter("kT", [D, T], BF16, isOutput=False)
    k8_in = nc.declare_dram_parameter("k8", [128, 2 * 2 * T], FP8, isOutput=False)
    qT_in = nc.declare_dram_parameter("qT", [D, T], BF16, isOutput=False)
    wq_in = nc.declare_dram_parameter("wq", [D, GC], BF16, isOutput=False)
    wk_in = nc.declare_dram_parameter("wk", [128, 2 * 2 * GC], FP8, isOutput=False)
    wv_in = nc.declare_dram_parameter("wv", [D, GC], BF16, isOutput=False)
    oT_out = nc.declare_dram_parameter("oT", [GC, T], F32, isOutput=True)

    with tile.TileContext(nc) as tc:
        with (
            tc.tile_pool(name="stage", bufs=4) as stage,
            tc.tile_pool(name="const", bufs=1) as const,
            tc.tile_pool(name="act", bufs=1) as actp,
            tc.tile_pool(name="pt", bufs=2) as ptp,
            tc.tile_pool(name="small", bufs=4) as small,
            tc.tile_pool(name="scr", bufs=3, space="PSUM") as scr,
            tc.tile_pool(name="psP", bufs=2, space="PSUM") as psP,
        ):
            # ---- weights + staging; DMA order gates the ramp:
            # wk8 -> kin8 (k-proj can start) -> wq -> qT tb0 (first q-proj)
            # -> wv -> remaining qT -> kT (v-proj inputs, needed mid-slot-0)
            wk8 = const.tile([128, 2, 2, GC], FP8, tag="wk8")
            nc.sync.dma_start(wk8[:], wk_in.rearrange("p (m j c) -> p m j c", m=2, j=2))
            kin8 = actp.tile([128, 2, 2, T], FP8, tag="kin8", name="kin8")
            # tb-chunked so the first k-proj chains start before the full
            # 1MB transfer lands
            for tb in range(NQB):
                nc.sync.dma_start(
                    kin8[:, :, :, tb * QB : (tb + 1) * QB],
                    k8_in.rearrange("p (m j c) -> p m j c", m=2, j=2)[
                        :, :, :, tb * QB : (tb + 1) * QB
                    ],
                )
            wq = const.tile([128, KS, GC], BF16, tag="wq")
            nc.sync.dma_start(wq[:], wq_in.rearrange("(s p) c -> p s c", p=128))
            qin = [
                stage.tile([128, T], BF16, tag="qkin", name=f"qin{s}")
                for s in range(KS)
            ]
            kin = [
                stage.tile([128, T], BF16, tag="kkin", name=f"kin{s}")
                for s in range(KS)
            ]
            for s in range(KS):
                nc.sync.dma_start(
                    qin[s][:, 0:QB], qT_in[s * 128 : (s + 1) * 128, 0:QB]
                )
            wv = const.tile([128, KS, GC], BF16, tag="wv")
            nc.sync.dma_start(wv[:], wv_in.rearrange("(s p) c -> p s c", p=128))
            for tb in range(1, NQB):
                for s in range(KS):
                    nc.sync.dma_start(
                        qin[s][:, tb * QB : (tb + 1) * QB],
                        qT_in[s * 128 : (s + 1) * 128, tb * QB : (tb + 1) * QB],
                    )
            for tb in range(NQB):
                for s in range(KS):
                    nc.sync.dma_start(
                        kin[s][:, tb * QB : (tb + 1) * QB],
                        kT_in[s * 128 : (s + 1) * 128, tb * QB : (tb + 1) * QB],
                    )

            # ---- warm the exp activation table early ----
            warm = small.tile([1, 8], F32, tag="warm", name="warm")
            nc.gpsimd.memset(warm[:], 0.0)
            nc.scalar.activation(warm[:], warm[:], EXP)

            kT_att = [
                actp.tile([128, T], BF16, tag=f"ka{t2}", name=f"ka{t2}")
                for t2 in range(2)
            ]
            qT_att = [
                actp.tile([128, T], BF16, tag=f"qa{t2}", name=f"qa{t2}")
                for t2 in range(2)
            ]

            def emit_kproj(t2, tb):
                ps = scr.tile([128, 2 * QB], F32, tag="scr", name="kproj_ps")
                for m in range(2):
                    nc.tensor.matmul(
                        ps[:, 0:QB],
                        wk8[:, m, :, t2 * 128 : (t2 + 1) * 128],
                        kin8[:, m, :, tb * QB : (tb + 1) * QB],
                        start=(m == 0),
                        stop=(m == 1),
                        perf_mode=DR,
                    )
                nc.vector.tensor_copy(
                    kT_att[t2][:, tb * QB : (tb + 1) * QB], ps[:, 0:QB]
                )

            def emit_qproj(t2, tb):
                ps = scr.tile([128, 2 * QB], F32, tag="scr", name="qproj_ps")
                for s in range(KS):
                    nc.tensor.matmul(
                        ps[:, 0:QB],
                        wq[:, s, t2 * 128 : (t2 + 1) * 128],
                        qin[s][:, tb * QB : (tb + 1) * QB],
                        start=(s == 0),
                        stop=(s == KS - 1),
                    )
                nc.vector.tensor_copy(
                    qT_att[t2][:, tb * QB : (tb + 1) * QB], ps[:, 0:QB]
                )

            # ---- V projection into [128, kt, head, 65] with ones column ----
            vp = const.tile([128, NKT, HPC, DH + 1], BF16, tag="vp")
            ones_f32 = const.tile([128, NKT * HPC], F32, tag="ones")
            nc.gpsimd.memset(ones_f32[:], 1.0)
            nc.vector.tensor_copy(
                vp[:, :, :, DH : DH + 1],
                ones_f32[:].rearrange("p (a b) -> p a b", b=HPC).unsqueeze(3),
            )

            def emit_vproj(tt):
                ps = scr.tile([128, 2 * QB], F32, tag="scr", name="vproj_ps")
                for s in range(KS):
                    nc.tensor.matmul(
                        ps[:, 0:GC],
                        kin[s][:, tt * 128 : (tt + 1) * 128],
                        wv[:, s, :],
                        start=(s == 0),
                        stop=(s == KS - 1),
                    )
                nc.vector.tensor_copy(
                    vp[:, tt, :, 0:DH],
                    ps[:, 0:GC].rearrange("p (h d) -> p h d", d=DH),
                )

            # ---- attention helpers ----
            def emit_score_group(qb, t2, ptile, kt):
                for hp in range(2):
                    base = 64 * hp
                    nc.tensor.matmul(
                        ptile[:, hp * QB : (hp + 1) * QB],
                        kT_att[t2][base : base + DH, kt * 128 : (kt + 1) * 128],
                        qT_att[t2][base : base + DH, qb * QB : (qb + 1) * QB],
                        start=True,
                        stop=True,
                        tile_position=(base, 0),
                    )

            def emit_exp(ptile, pt, kt, engine):
                out = pt[:, kt, :]
                in_ = ptile[:]
                if engine == "S":
                    nc.scalar.activation(out, in_, EXP, scale=SCALE)
                else:
                    nc.vector._custom_dve(
                        EXP_OP, out=out, in0=in_, s0=PC3, s1=PC2, imm2=PC1
                    )

            def emit_av_chunk(prev, kt_lo, kt_hi):
                qb, t2, pt, po = prev
                for kt in range(kt_lo, kt_hi):
                    for hp in range(2):
                        nc.tensor.matmul(
                            po[hp][0 : DH + 1],
                            vp[:, kt, 2 * t2 + hp, :],
                            pt[:, kt, hp * QB : (hp + 1) * QB],
                            start=(kt == 0),
                            stop=(kt == NKT - 1),
                        )

            def emit_norm(prev):
                # the two heads' chains are emitted stage-interleaved so they
                # pipeline across Vector/GpSimd instead of running serially
                # (matters at the kernel tail, where nothing else overlaps)
                qb, t2, pt, po = prev
                sums, rec, bc, ot = [], [], [], []
                for hp in range(2):
                    sums.append(small.tile([1, QB], F32, tag="sums", name="sums"))
                    nc.vector.tensor_copy(sums[hp][:], po[hp][DH : DH + 1, :])
                for hp in range(2):
                    rec.append(small.tile([1, QB], F32, tag="rec", name="rec"))
                    nc.vector.reciprocal_approx_fast(rec[hp][:], sums[hp][:])
                for hp in range(2):
                    bc.append(small.tile([DH, QB], F32, tag="bc", name="bc"))
                    nc.gpsimd.partition_broadcast(bc[hp][:], rec[hp][:])
                for hp in range(2):
                    ot.append(small.tile([DH, QB], F32, tag="ot", name="ot"))
                    nc.vector.tensor_mul(ot[hp][:], po[hp][0:DH, :], bc[hp][:])
                for hp in range(2):
                    habs = 2 * t2 + hp
                    nc.gpsimd.dma_start(
                        oT_out[habs * DH : (habs + 1) * DH, qb * QB : (qb + 1) * QB],
                        ot[hp][:],
                    )

            # ---- prologue: just enough projection for slot 0 ----
            for tb in range(NQB):
                emit_kproj(0, tb)
            emit_qproj(0, 0)

            # filler projection chains, paced one per score group
            slot_fillers = {
                0: [lambda tb=tb: emit_kproj(1, tb) for tb in range(NQB)]
                + [lambda: emit_qproj(1, 0)]
                + [lambda tt=tt: emit_vproj(tt) for tt in range(0, 10)],
                1: [lambda tt=tt: emit_vproj(tt) for tt in range(10, NKT)]
                + [
                    lambda: emit_qproj(0, 1),
                    lambda: emit_qproj(1, 1),
                ],
                2: [lambda: emit_qproj(0, 2)],
                3: [lambda: emit_qproj(1, 2)],
                4: [lambda: emit_qproj(0, 3)],
                5: [lambda: emit_qproj(1, 3)],
            }

            slots = [(qb, t2) for qb in range(NQB) for t2 in range(2)]
            prev = None
            last = None
            for si, (qb, t2) in enumerate(slots):
                is_last = si == len(slots) - 1
                fillers = slot_fillers.get(si, [])
                pt = ptp.tile([128, NKT, 2 * QB], BF16, tag="pt", name="pt")
                if prev is not None:
                    po = [
                        psP.tile([128, QB], F32, tag="P", name=f"po{hp}")
                        for hp in range(2)
                    ]
                    prev = (*prev, po)
                for kt in range(NKT):
                    if prev is not None and kt in AV_BEFORE:
                        emit_av_chunk(prev, *AV_BEFORE[kt])
                        if kt == 12:
                            # norm(prev) right after its last AV chunk so its
                            # DVE ops don't queue behind this slot's late exps
                            emit_norm(prev)
                    ptile = scr.tile([128, 2 * QB], F32, tag="scr", name="s_ps")
                    emit_score_group(qb, t2, ptile, kt)
                    if fillers:
                        fillers.pop(0)()
                    emit_exp(ptile, pt, kt, ENGINES[kt])
                    if is_last and kt == 13:
                        # own-AV po lives in a scr-pool tile (the psP pair is
                        # still held by prev until its norm) so the final
                        # slot's AV drains inline instead of serially after
                        # the loop; allocated here so the remaining two score
                        # groups reuse the kt12/kt13 bufs, not this one
                        own = scr.tile([128, 2 * QB], F32, tag="scr", name="own_po")
                        last = (qb, t2, pt, [own[:, 0:QB], own[:, QB : 2 * QB]])
                        emit_av_chunk(last, 0, 12)
                while fillers:
                    fillers.pop(0)()
                prev = (qb, t2, pt)
            emit_av_chunk(last, 12, NKT)
            emit_norm(last)

    nc.compile()
    return nc


_NC = None


def _get_nc():
    global _NC
    if _NC is None:
        _NC = build()
    return _NC


def _dr_pack(x):
    """[512, C] f32 -> fp8 [128, 2, 2, C] with slab j of pair m = slice 2m+j."""
    x8 = x.astype(ml_dtypes.float8_e4m3)
    return np.ascontiguousarray(
        x8.reshape(2, 2, 128, -1).transpose(2, 0, 1, 3).reshape(128, -1)
    )


def run(query, key, W_query, W_key, W_value, trace=False):
    nc = _get_nc()
    query = np.asarray(query, dtype=np.float32)
    key = np.asarray(key, dtype=np.float32)
    W_query = np.asarray(W_query, dtype=np.float32)
    W_key = np.asarray(W_key, dtype=np.float32)
    W_value = np.asarray(W_value, dtype=np.float32)

    in_maps = []
    for c in range(8):
        n, g = c // 2, c % 2
        cols = slice(g * GC, (g + 1) * GC)
        in_maps.append(
            {
                "kT": np.ascontiguousarray(key[n].T.astype(ml_dtypes.bfloat16)),
                "k8": _dr_pack(key[n].T),
                "qT": np.ascontiguousarray(query[n].T.astype(ml_dtypes.bfloat16)),
                "wq": np.ascontiguousarray(W_query[:, cols].astype(ml_dtypes.bfloat16)),
                "wk": _dr_pack(W_key[:, cols]),
                "wv": np.ascontiguousarray(W_value[:, cols].astype(ml_dtypes.bfloat16)),
            }
        )
    res = run_bass_kernel_spmd(nc, in_maps, core_ids=list(range(8)), trace=trace)
    out = np.empty((N, T, D), dtype=np.float32)
    for c in range(8):
        n, g = c // 2, c % 2
        out[n, :, g * GC : (g + 1) * GC] = res.results[c]["oT"].T
    return out, res


def kernel(query, key, W_query, W_key, W_value):
    out, _ = run(query, key, W_query, W_key, W_value, trace=False)
    return out


# revision 52
# speedup vs baseline: 1.0317x; 1.0317x over previous
"""Multi-head attention (N=4, T=2048, D=512, H=8, dh=64) on 8 TRN2 NeuronCores.

Sharding: batch N (4) x head-group (2 groups of 4 heads) -> 8 cores.
Per core, for its (batch n, head-group g) the kernel computes
  q = query[n] @ Wq[:, 256g:+256], k/v likewise, then per head
  softmax(q k^T / sqrt(512)) v, assembled host-side from oT tiles.

Implementation notes:
- Score matmuls (contraction dh=64) alternate the two heads of a pair
  between PE row-tiles (0,0)/(64,0); the tiles stream concurrently
  (~227ns per matmul pair vs ~490ns/matmul same-tile, HW-measured).
- q/k projections run as fp8(e4m3) DoubleRow matmuls: contraction 512 is
  consumed 256/pass (two interleaved 128-row slabs), halving the chain.
  Only the scores depend on q/k, and softmax normalization makes the
  score path tolerant of fp8 (|s|<0.8, exp flat). V stays bf16.
- exp() is split across ScalarE ACTIVATE (true exp) and a custom DVE op
  evaluating a minimax cubic (c0=1, rel err 6e-3 on +-0.9; scores are
  tiny so no max-subtraction is needed and the cubic stays positive).
- PSUM: three 2-bank score pools rotate (depth 3) so the score matmuls
  never wait on a single exp; 16 one-ktile groups per (q-block,
  head-pair) slot, engines interleaved S/D. The previous slot's AV
  matmuls (bf16, ones-column for denominators) fill the gaps at a fixed
  cadence, its normalize runs right after the last AV chunk, and the
  final slot drains its own AV inline to shorten the tail.
"""

import math

import ml_dtypes
import numpy as np

import concourse.bass as bass
import concourse.mybir as mybir
import concourse.tile as tile
from concourse import bacc
from concourse.bass_utils import run_bass_kernel_spmd

# ---- custom DVE op: out = 1 + x(c1 + x(c2 + x c3)) ------------------------
import concourse.dve_ops as dve_ops
from concourse.dve_spec import Spec, Src0, C0, C1, C2, One, lower
from concourse.dve_uop import DveOpSpec


def _register_exp_op():
    name = "EXP_POLY3_ANT"
    for o in dve_ops.OPS:
        if o.name == name:
            return o
    body = ((Src0 * C0 + C1) * Src0 + C2) * Src0 + One
    spec = Spec(
        body=body,
        reference=lambda in0, s0, s1, imm2: ((in0 * s0 + s1) * in0 + imm2) * in0
        + 1.0,
    )
    row = dve_ops._CUSTOM_DVE_ROW_BASE + len(dve_ops.OPS)
    shas = {}
    for ver in ("v3", "v4"):
        try:
            uops = lower(spec, ver=ver)
            shas[ver] = DveOpSpec(
                name=name, opcode=row, uops=uops, rd1_en=False
            ).sha(ver)
        except Exception:
            pass
    op = dve_ops.DveOp(name, spec, subdim=False, uops_sha=shas)
    dve_ops.OPS.append(op)
    dve_ops._SUB_OPCODE_FOR_NAME[name] = row
    dve_ops.CUSTOM_DVE_SPECS[name] = spec
    return op


EXP_OP = _register_exp_op()

F32 = mybir.dt.float32
BF16 = mybir.dt.bfloat16
FP8 = mybir.dt.float8e4
EXP = mybir.ActivationFunctionType.Exp
DR = mybir.MatmulPerfMode.DoubleRow

N, T, D = 4, 2048, 512
HPC, DH = 4, 64          # heads per core, head dim
GC = HPC * DH            # head-group columns (256)
SCALE = 1.0 / math.sqrt(D)
QB = 512                 # q block
NQB = T // QB            # 4
NKT = T // 128           # 16 k tiles
KS = D // 128            # 4 contraction slices for projections

# minimax cubic for exp on [-0.9, 0.9], c0 = 1; coeffs pre-scaled by SCALE^i
_C1, _C2, _C3 = 1.0122206024824583, 0.5302855202358088, 0.15680354230475546
PC1, PC2, PC3 = _C1 * SCALE, _C2 * SCALE**2, _C3 * SCALE**3

# per-slot exp engine pattern over the 16 one-ktile groups (S=ScalarE true
# exp, D=DVE cubic); 10 S / 6 D balances the two engines' throughput.
ENGINES = "SDSSDSDSSDSDSSDS"
# AV chunks for the previous slot, emitted before the group at these
# indices (the chunk at 0 covers the slot-start pool/exp lag).
AV_BEFORE = {0: (0, 4), 3: (4, 7), 6: (7, 10), 9: (10, 13), 12: (13, 16)}


def build():
    nc = bacc.Bacc("TRN2", target_bir_lowering=False, debug=False, num_devices=8)
    kT_in = nc.declare_dram_parameter("kT", [D, T], BF16, isOutput=False)
    k8_in = nc.declare_dram_parameter("k8", [128, 2 * 2 * T], FP8, isOutput=False)
    qT_in = nc.declare_dram_parameter("qT", [D, T], BF16, isOutput=False)
    wq_in = nc.declare_dram_parameter("wq", [D, GC], BF16, isOutput=False)
    wk_in = nc.declare_dram_parameter("wk", [128, 2 * 2 * GC], FP8, isOutput=False)
    wv_in = nc.declare_dram_parameter("wv", [D, GC], BF16, isOutput=False)
    oT_out = nc.declare_dram_parameter("oT", [GC, T], F32, isOutput=True)

    with tile.TileContext(nc) as tc:
        with (
            tc.tile_pool(name="stage", bufs=4) as stage,
            tc.tile_pool(name="const", bufs=1) as const,
            tc.tile_pool(name="act", bufs=1) as actp,
            tc.tile_pool(name="pt", bufs=2) as ptp,
            tc.tile_pool(name="small", bufs=4) as small,
            tc.tile_pool(name="scr", bufs=3, space="PSUM") as scr,
            tc.tile_pool(name="psP", bufs=2, space="PSUM") as psP,
        ):
            # ---- weights + staging; DMA order gates the ramp:
            # wk8 -> kin8 (k-proj can start) -> wq -> qT tb0 (first q-proj)
            # -> wv -> remaining qT -> kT (v-proj inputs, needed mid-slot-0)
            wk8 = const.tile([128, 2, 2, GC], FP8, tag="wk8")
            nc.sync.dma_start(wk8[:], wk_in.rearrange("p (m j c) -> p m j c", m=2, j=2))
            kin8 = actp.tile([128, 2, 2, T], FP8, tag="kin8", name="kin8")
            nc.sync.dma_start(kin8[:], k8_in.rearrange("p (m j c) -> p m j c", m=2, j=2))
            wq = const.tile([128, KS, GC], BF16, tag="wq")
            nc.sync.dma_start(wq[:], wq_in.rearrange("(s p) c -> p s c", p=128))
            qin = [
                stage.tile([128, T], BF16, tag="qkin", name=f"qin{s}")
                for s in range(KS)
            ]
            kin = [
                stage.tile([128, T], BF16, tag="kkin", name=f"kin{s}")
                for s in range(KS)
            ]
            for s in range(KS):
                nc.sync.dma_start(
                    qin[s][:, 0:QB], qT_in[s * 128 : (s + 1) * 128, 0:QB]
                )
            wv = const.tile([128, KS, GC], BF16, tag="wv")
            nc.sync.dma_start(wv[:], wv_in.rearrange("(s p) c -> p s c", p=128))
            for tb in range(1, NQB):
                for s in range(KS):
                    nc.sync.dma_start(
                        qin[s][:, tb * QB : (tb + 1) * QB],
                        qT_in[s * 128 : (s + 1) * 128, tb * QB : (tb + 1) * QB],
                    )
            for tb in range(NQB):
                for s in range(KS):
                    nc.sync.dma_start(
                        kin[s][:, tb * QB : (tb + 1) * QB],
                        kT_in[s * 128 : (s + 1) * 128, tb * QB : (tb + 1) * QB],
                    )

            # ---- warm the exp activation table early ----
            warm = small.tile([1, 8], F32, tag="warm", name="warm")
            nc.gpsimd.memset(warm[:], 0.0)
            nc.scalar.activation(warm[:], warm[:], EXP)

            kT_att = [
                actp.tile([128, T], BF16, tag=f"ka{t2}", name=f"ka{t2}")
                for t2 in range(2)
            ]
            qT_att = [
                actp.tile([128, T], BF16, tag=f"qa{t2}", name=f"qa{t2}")
                for t2 in range(2)
            ]

            def emit_kproj(t2, tb):
                ps = scr.tile([128, 2 * QB], F32, tag="scr", name="kproj_ps")
                for m in range(2):
                    nc.tensor.matmul(
                        ps[:, 0:QB],
                        wk8[:, m, :, t2 * 128 : (t2 + 1) * 128],
                        kin8[:, m, :, tb * QB : (tb + 1) * QB],
                        start=(m == 0),
                        stop=(m == 1),
                        perf_mode=DR,
                    )
                nc.vector.tensor_copy(
                    kT_att[t2][:, tb * QB : (tb + 1) * QB], ps[:, 0:QB]
                )

            def emit_qproj(t2, tb):
                ps = scr.tile([128, 2 * QB], F32, tag="scr", name="qproj_ps")
                for s in range(KS):
                    nc.tensor.matmul(
                        ps[:, 0:QB],
                        wq[:, s, t2 * 128 : (t2 + 1) * 128],
                        qin[s][:, tb * QB : (tb + 1) * QB],
                        start=(s == 0),
                        stop=(s == KS - 1),
                    )
                nc.vector.tensor_copy(
                    qT_att[t2][:, tb * QB : (tb + 1) * QB], ps[:, 0:QB]
                )

            # ---- V projection into [128, kt, head, 65] with ones column ----
            vp = const.tile([128, NKT, HPC, DH + 1], BF16, tag="vp")
            ones_f32 = const.tile([128, NKT * HPC], F32, tag="ones")
            nc.gpsimd.memset(ones_f32[:], 1.0)
            nc.vector.tensor_copy(
                vp[:, :, :, DH : DH + 1],
                ones_f32[:].rearrange("p (a b) -> p a b", b=HPC).unsqueeze(3),
            )

            def emit_vproj(tt):
                ps = scr.tile([128, 2 * QB], F32, tag="scr", name="vproj_ps")
                for s in range(KS):
                    nc.tensor.matmul(
                        ps[:, 0:GC],
                        kin[s][:, tt * 128 : (tt + 1) * 128],
                        wv[:, s, :],
                        start=(s == 0),
                        stop=(s == KS - 1),
                    )
                nc.vector.tensor_copy(
                    vp[:, tt, :, 0:DH],
                    ps[:, 0:GC].rearrange("p (h d) -> p h d", d=DH),
                )

            # ---- attention helpers ----
            def emit_score_group(qb, t2, ptile, kt):
                for hp in range(2):
                    base = 64 * hp
                    nc.tensor.matmul(
                        ptile[:, hp * QB : (hp + 1) * QB],
                        kT_att[t2][base : base + DH, kt * 128 : (kt + 1) * 128],
                        qT_att[t2][base : base + DH, qb * QB : (qb + 1) * QB],
                        start=True,
                        stop=True,
                        tile_position=(base, 0),
                    )

            def emit_exp(ptile, pt, kt, engine):
                out = pt[:, kt, :]
                in_ = ptile[:]
                if engine == "S":
                    nc.scalar.activation(out, in_, EXP, scale=SCALE)
                else:
                    nc.vector._custom_dve(
                        EXP_OP, out=out, in0=in_, s0=PC3, s1=PC2, imm2=PC1
                    )

            def emit_av_chunk(prev, kt_lo, kt_hi):
                qb, t2, pt, po = prev
                for kt in range(kt_lo, kt_hi):
                    for hp in range(2):
                        nc.tensor.matmul(
                            po[hp][0 : DH + 1],
                            vp[:, kt, 2 * t2 + hp, :],
                            pt[:, kt, hp * QB : (hp + 1) * QB],
                            start=(kt == 0),
                            stop=(kt == NKT - 1),
                        )

            def emit_norm(prev):
                qb, t2, pt, po = prev
                for hp in range(2):
                    habs = 2 * t2 + hp
                    sums = small.tile([1, QB], F32, tag="sums", name="sums")
                    nc.vector.tensor_copy(sums[:], po[hp][DH : DH + 1, :])
                    rec = small.tile([1, QB], F32, tag="rec", name="rec")
                    nc.vector.reciprocal_approx_fast(rec[:], sums[:])
                    bc = small.tile([DH, QB], F32, tag="bc", name="bc")
                    nc.gpsimd.partition_broadcast(bc[:], rec[:])
                    ot = small.tile([DH, QB], F32, tag="ot", name="ot")
                    nc.vector.tensor_mul(ot[:], po[hp][0:DH, :], bc[:])
                    nc.gpsimd.dma_start(
                        oT_out[habs * DH : (habs + 1) * DH, qb * QB : (qb + 1) * QB],
                        ot[:],
                    )

            # ---- prologue: just enough projection for slot 0 ----
            for tb in range(NQB):
                emit_kproj(0, tb)
            emit_qproj(0, 0)

            # filler projection chains, paced one per score group
            slot_fillers = {
                0: [lambda tb=tb: emit_kproj(1, tb) for tb in range(NQB)]
                + [lambda: emit_qproj(1, 0)]
                + [lambda tt=tt: emit_vproj(tt) for tt in range(0, 10)],
                1: [lambda tt=tt: emit_vproj(tt) for tt in range(10, NKT)]
                + [
                    lambda: emit_qproj(0, 1),
                    lambda: emit_qproj(1, 1),
                ],
                2: [lambda: emit_qproj(0, 2)],
                3: [lambda: emit_qproj(1, 2)],
                4: [lambda: emit_qproj(0, 3)],
                5: [lambda: emit_qproj(1, 3)],
            }

            slots = [(qb, t2) for qb in range(NQB) for t2 in range(2)]
            prev = None
            last = None
            for si, (qb, t2) in enumerate(slots):
                is_last = si == len(slots) - 1
                fillers = slot_fillers.get(si, [])
                pt = ptp.tile([128, NKT, 2 * QB], BF16, tag="pt", name="pt")
                if prev is not None:
                    po = [
                        psP.tile([128, QB], F32, tag="P", name=f"po{hp}")
                        for hp in range(2)
                    ]
                    prev = (*prev, po)
                for kt in range(NKT):
                    if prev is not None and kt in AV_BEFORE:
                        emit_av_chunk(prev, *AV_BEFORE[kt])
                        if kt == 12:
                            # norm(prev) right after its last AV chunk so its
                            # DVE ops don't queue behind this slot's late exps
                            emit_norm(prev)
                    ptile = scr.tile([128, 2 * QB], F32, tag="scr", name="s_ps")
                    emit_score_group(qb, t2, ptile, kt)
                    if fillers:
                        fillers.pop(0)()
                    emit_exp(ptile, pt, kt, ENGINES[kt])
                    if is_last and kt == 13:
                        # own-AV po lives in a scr-pool tile (the psP pair is
                        # still held by prev until its norm) so the final
                        # slot's AV drains inline instead of serially after
                        # the loop; allocated here so the remaining two score
                        # groups reuse the kt12/kt13 bufs, not this one
                        own = scr.tile([128, 2 * QB], F32, tag="scr", name="own_po")
                        last = (qb, t2, pt, [own[:, 0:QB], own[:, QB : 2 * QB]])
                        emit_av_chunk(last, 0, 12)
                while fillers:
                    fillers.pop(0)()
                prev = (qb, t2, pt)
            emit_av_chunk(last, 12, NKT)
            # final normalize: stage-interleave the two heads' chains so they
            # pipeline across Vector/GpSimd — at the kernel tail nothing else
            # overlaps them (mid-slot norms stay sequential: interleaving
            # there measured slower due to DVE burstiness)
            qb_l, t2_l, pt_l, po_l = last
            sums_l, rec_l, bc_l, ot_l = [], [], [], []
            for hp in range(2):
                sums_l.append(small.tile([1, QB], F32, tag="sums", name="sums"))
                nc.vector.tensor_copy(sums_l[hp][:], po_l[hp][DH : DH + 1, :])
            for hp in range(2):
                rec_l.append(small.tile([1, QB], F32, tag="rec", name="rec"))
                nc.vector.reciprocal_approx_fast(rec_l[hp][:], sums_l[hp][:])
            for hp in range(2):
                bc_l.append(small.tile([DH, QB], F32, tag="bc", name="bc"))
                nc.gpsimd.partition_broadcast(bc_l[hp][:], rec_l[hp][:])
            for hp in range(2):
                ot_l.append(small.tile([DH, QB], F32, tag="ot", name="ot"))
                nc.vector.tensor_mul(ot_l[hp][:], po_l[hp][0:DH, :], bc_l[hp][:])
            for hp in range(2):
                habs = 2 * t2_l + hp
                nc.gpsimd.dma_start(
                    oT_out[habs * DH : (habs + 1) * DH, qb_l * QB : (qb_l + 1) * QB],
                    ot_l[hp][:],
                )

    nc.compile()
    return nc


_NC = None


def _get_nc():
    global _NC
    if _NC is None:
        _NC = build()
    return _NC


def _dr_pack(x):
    """[512, C] f32 -> fp8 [128, 2, 2, C] with slab j of pair m = slice 2m+j."""
    x8 = x.astype(ml_dtypes.float8_e4m3)
    return np.ascontiguousarray(
        x8.reshape(2, 2, 128, -1).transpose(2, 0, 1, 3).reshape(128, -1)
    )


def run(query, key, W_query, W_key, W_value, trace=False):
    nc = _get_nc()
    query = np.asarray(query, dtype=np.float32)
    key = np.asarray(key, dtype=np.float32)
    W_query = np.asarray(W_query, dtype=np.float32)
    W_key = np.asarray(W_key, dtype=np.float32)
    W_value = np.asarray(W_value, dtype=np.float32)

    in_maps = []
    for c in range(8):
        n, g = c // 2, c % 2
        cols = slice(g * GC, (g + 1) * GC)
        in_maps.append(
            {
                "kT": np.ascontiguousarray(key[n].T.astype(ml_dtypes.bfloat16)),
                "k8": _dr_pack(key[n].T),
                "qT": np.ascontiguousarray(query[n].T.astype(ml_dtypes.bfloat16)),
                "wq": np.ascontiguousarray(W_query[:, cols].astype(ml_dtypes.bfloat16)),
                "wk": _dr_pack(W_key[:, cols]),
                "wv": np.ascontiguousarray(W_value[:, cols].astype(ml_dtypes.bfloat16)),
            }
        )
    res = run_bass_kernel_spmd(nc, in_maps, core_ids=list(range(8)), trace=trace)
    out = np.empty((N, T, D), dtype=np.float32)
    for c in range(8):
        n, g = c // 2, c % 2
        out[n, :, g * GC : (g + 1) * GC] = res.results[c]["oT"].T
    return out, res


def kernel(query, key, W_query, W_key, W_value):
    out, _ = run(query, key, W_query, W_key, W_value, trace=False)
    return out
